# revision 16
# baseline (speedup 1.0000x reference)
"""Trainium2 Bass kernel for nn_RankingLoss (pairwise hinge ranking loss).

reference semantics (N = 8192):
    d = targets[:,0]; e = targets[:,1]
    valid[i,j] = (d[i] < d[j]) & (e[i] == 1)
    hinge[i,j] = relu(1.0 - (p[i] - p[j]))
    loss = sum(valid*hinge) / max(sum(valid), 1)   (0 if no pairs)

Device algorithm (per core, j-axis sharded across 8 cores):
  Layout: partition axis = j (128 per tile, 8 tiles per core), free axis = i
  (full 8192, processed in 4 macro-chunks of 2048).  The i-axis vectors are
  broadcast across the 128 partitions with a K=1 TensorE outer product
  (ones[1,128].T @ row[1,n] -> PSUM) + engine copies; this keeps every
  hot-loop dependency a single-engine semaphore (walrus fits only one sync
  wait on LDWEIGHTS, so DMA-queue fanout into compute ops must be avoided).

  A_e[j,i] = [dmask_i < d_j]            dmask_i = e_i ? d_i : 1e6
             (events-mask and duration compare fused; ScalarE sigmoid
              (BIG*(d_j - dmask_i)) or DVE tensor_scalar is_lt; accum_out
              gives R = per-j num_pairs partial)
  J[j,i]   = [p_i < p_j + 1] * A_e      one fused scalar_tensor_tensor (DVE)
  PSUM     = sum_j J * [p_hi_j, p_lo_j, 1]   via TensorE matmuls, col-tiled:
             i-subchunk s of 4 accumulates at psum partitions 32s..32s+2 of
             one bank (tile_position=(0,32s)), so a whole macro-chunk's
             accumulator is a single fresh psum bank (p_hi+p_lo = f32 preds
             split into two bf16 for precision).

  Host combines:  S1e_i = rows0+1, S0e_i = row2 (pre-weighted by e_i through
  dmask), loss_sum = sum_i S1e + (1-p_i)*S0e, num_pairs = sum(R).
  The [p_i < p_j+1] compare runs in bf16: any pair it can misclassify has
  |hinge| <= one bf16 ulp, so loss error stays ~1e-4 relative; the duration
  compare is exact f32 (sigmoid boundary errors need |d_i-d_j| < ~1e-7*d).
"""

import numpy as np
import ml_dtypes

N = 8192
NCORES = 8
JB = N // NCORES          # j-block per core = 1024
NT = JB // 128            # j-tiles per core = 8
CH = 2048                 # i macro-chunk width
NCH = N // CH             # 4
SUB = 512                 # matmul N / psum bank width (f32)
NSUB = CH // SUB          # 4
BCH = 1024                # broadcast psum chunk width
BIG = np.float32(1.0e30)
# Finite sentinel > any real duration; BIG * DMASK_FILL stays finite in f32.
DMASK_FILL = np.float32(1.0e6)
BF16 = ml_dtypes.bfloat16

# A-op engine assignment by j-tile index t: these run on ScalarE (sigmoid),
# these on GpSimd (is_lt), the rest on DVE (is_lt).
ACT_T = (0, 1, 2, 3, 4)
GP_T = ()

_CACHE = {}


def _build_module():
    import concourse.bass as bass
    import concourse.bacc as bacc
    import concourse.tile as tile
    from concourse import mybir

    f32 = mybir.dt.float32
    bf16 = mybir.dt.bfloat16
    Alu = mybir.AluOpType
    Act = mybir.ActivationFunctionType

    n_act = len(ACT_T) * NCH
    n_gp = len(GP_T) * NCH
    n_dve = (NT - len(ACT_T) - len(GP_T)) * NCH

    nc = bacc.Bacc(trn_type="TRN2")
    t_dmask = nc.dram_tensor("dmask", [N], f32, kind="ExternalInput")
    t_pbf = nc.dram_tensor("predsbf", [N], bf16, kind="ExternalInput")
    # djcols: [:, 0:NT] = dj, [:, NT:2NT] = BIG*dj
    t_djcols = nc.dram_tensor("djcols", [128, 2 * NT], f32, kind="ExternalInput")
    # pcols: [:, 0:3NT] = lhst ([p_hi|p_lo|1] per t), [:, 3NT:4NT] = bf16(p_j+1)
    t_pcols = nc.dram_tensor("pcols", [128, 4 * NT], bf16, kind="ExternalInput")
    t_outj = nc.dram_tensor("outj", [NCH, NSUB, 3, SUB], f32, kind="ExternalOutput")
    t_outra = nc.dram_tensor("outra", [128, max(n_act, 1)], f32, kind="ExternalOutput")
    t_outrg = nc.dram_tensor("outrg", [128, max(n_gp, 1)], f32, kind="ExternalOutput")
    t_outrd = nc.dram_tensor("outrd", [128, max(n_dve, 1)], f32, kind="ExternalOutput")

    with tile.TileContext(nc) as tc:
        with (
            tc.tile_pool(name="consts", bufs=1) as consts,
            tc.tile_pool(name="bcast", bufs=1) as bcast,
            tc.tile_pool(name="awork", bufs=3) as awork,
            tc.tile_pool(name="jwork", bufs=3) as jwork,
            tc.tile_pool(name="stage", bufs=2) as stagep,
            tc.tile_pool(name="scratch", bufs=1) as scratch,
            tc.tile_pool(name="bps", bufs=2, space="PSUM") as bpsp,
            tc.tile_pool(name="acc", bufs=4, space="PSUM") as accp,
        ):
            djcols_s = consts.tile([128, 2 * NT], f32, tag="djcols")
            pcols_s = consts.tile([128, 4 * NT], bf16, tag="pcols")
            drow = consts.tile([1, N], f32, tag="drow")
            prow = consts.tile([1, N], bf16, tag="prow")
            ones_f = consts.tile([1, 128], f32, tag="ones_f")
            ones_b = consts.tile([1, 128], bf16, tag="ones_b")
            r_act = consts.tile([128, max(n_act, 1)], f32, tag="ract")
            r_gp = consts.tile([128, max(n_gp, 1)], f32, tag="rgp")
            r_dve = consts.tile([128, max(n_dve, 1)], f32, tag="rdve")

            nc.sync.dma_start(djcols_s[:], t_djcols[:])
            nc.sync.dma_start(pcols_s[:], t_pcols[:])
            nc.sync.dma_start(drow[:], t_dmask.rearrange("(a n) -> a n", a=1))
            nc.sync.dma_start(prow[:], t_pbf.rearrange("(a n) -> a n", a=1))
            nc.vector.memset(ones_f[:], 1.0)
            nc.vector.memset(ones_b[:], 1.0)
            if n_act == 0:
                nc.vector.memset(r_act[:], 0.0)
            if n_gp == 0:
                nc.vector.memset(r_gp[:], 0.0)
            if n_dve == 0:
                nc.vector.memset(r_dve[:], 0.0)

            # Tiny warm-up copies so the big ops don't accumulate DMA waits.
            warm_a = scratch.tile([128, 1], f32, tag="warm_a")
            warm_v = scratch.tile([128, 1], bf16, tag="warm_v")
            nc.scalar.copy(warm_a[:], djcols_s[:, 0:1])
            nc.vector.tensor_copy(warm_v[:], pcols_s[:, 0:1])
            if GP_T:
                warm_g = scratch.tile([128, 1], f32, tag="warm_g")
                nc.gpsimd.tensor_copy(warm_g[:], djcols_s[:, 0:1])

            # Broadcast i-axis vectors across partitions: PE outer product
            # (ones.T @ row) into PSUM, then engine copy to SBUF per chunk.
            # All d-chunks first (ScalarE copies), then p-chunks (DVE copies)
            # so every psum-slot reuse is gated by one engine semaphore.
            dbc = [
                bcast.tile([128, CH], f32, tag=f"dbc{C}", name=f"dbc{C}")
                for C in range(NCH)
            ]
            pbc = [
                bcast.tile([128, CH], bf16, tag=f"pbc{C}", name=f"pbc{C}")
                for C in range(NCH)
            ]
            first = True
            for C in range(NCH):
                for h in range(CH // BCH):
                    off = C * CH + h * BCH
                    bp = bpsp.tile([128, BCH], f32, tag="bps")
                    if first:
                        # Dummy 1x1 matmuls: advance PE's vector clock past
                        # the memsets and row DMAs one semaphore at a time
                        # (LDWEIGHTS fits a single sync wait).
                        for wlhs, wrhs in (
                            (ones_b, ones_b),
                            (ones_f, drow),
                            (ones_b, prow),
                        ):
                            nc.tensor.matmul(
                                bp[0:1, 0:1], wlhs[0:1, 0:1], wrhs[0:1, 0:1],
                                start=True, stop=True,
                            )
                        first = False
                    for s in range(BCH // SUB):
                        nc.tensor.matmul(
                            bp[:, s * SUB : (s + 1) * SUB],
                            ones_f[:],
                            drow[0:1, off + s * SUB : off + (s + 1) * SUB],
                            start=True,
                            stop=True,
                        )
                    nc.scalar.copy(dbc[C][:, h * BCH : (h + 1) * BCH], bp[:])
            for C in range(NCH):
                for h in range(CH // BCH):
                    off = C * CH + h * BCH
                    bp = bpsp.tile([128, BCH], f32, tag="bps")
                    for s in range(BCH // SUB):
                        nc.tensor.matmul(
                            bp[:, s * SUB : (s + 1) * SUB],
                            ones_b[:],
                            prow[0:1, off + s * SUB : off + (s + 1) * SUB],
                            start=True,
                            stop=True,
                        )
                    nc.vector.tensor_copy(pbc[C][:, h * BCH : (h + 1) * BCH], bp[:])

            ia = 0
            ig = 0
            iv = 0
            for C in range(NCH):
                ps_c = accp.tile([128, SUB], f32, tag="acc")
                nc.vector.memset(ps_c[:], 0.0)
                for t in range(NT):
                    a_t = awork.tile([128, CH], bf16, tag="a")
                    if t in ACT_T:
                        nc.scalar.activation(
                            a_t[:],
                            dbc[C][:],
                            Act.Sigmoid,
                            bias=djcols_s[:, NT + t : NT + t + 1],
                            scale=-float(BIG),
                            accum_out=r_act[:, ia : ia + 1],
                        )
                        ia += 1
                    elif t in GP_T:
                        nc.gpsimd.tensor_scalar(
                            a_t[:],
                            dbc[C][:],
                            djcols_s[:, t : t + 1],
                            None,
                            Alu.is_lt,
                            Alu.add,  # reduce op for accum_out
                            accum_out=r_gp[:, ig : ig + 1],
                        )
                        ig += 1
                    else:
                        nc.vector.tensor_scalar(
                            a_t[:],
                            dbc[C][:],
                            djcols_s[:, t : t + 1],
                            None,
                            Alu.is_lt,
                            Alu.add,
                            accum_out=r_dve[:, iv : iv + 1],
                        )
                        iv += 1
                    j_t = jwork.tile([128, CH], bf16, tag="j")
                    nc.vector.scalar_tensor_tensor(
                        j_t[:],
                        pbc[C][:],
                        pcols_s[:, 3 * NT + t : 3 * NT + t + 1],
                        a_t[:],
                        Alu.is_lt,
                        Alu.mult,
                    )
                    for s in range(NSUB):
                        nc.tensor.matmul(
                            ps_c[32 * s : 32 * s + 3, :],
                            pcols_s[:, 3 * t : 3 * t + 3],
                            j_t[:, s * SUB : (s + 1) * SUB],
                            start=(t == 0),
                            stop=(t == NT - 1),
                            tile_position=(0, 32 * s),
                        )
                st = stagep.tile([128, SUB], f32, tag="st")
                nc.scalar.copy(st[:], ps_c[:])
                for s in range(NSUB):
                    nc.sync.dma_start(
                        t_outj[C, s], st[32 * s : 32 * s + 3, :]
                    )

            nc.sync.dma_start(t_outra[:], r_act[:])
            nc.sync.dma_start(t_outrg[:], r_gp[:])
            nc.sync.dma_start(t_outrd[:], r_dve[:])

    nc.finalize()  # Bacc: legalizes sync waits (event semaphores) + compiles
    return nc


def get_module():
    if "nc" not in _CACHE:
        _CACHE["nc"] = _build_module()
    return _CACHE["nc"]


def make_in_maps(preds, targets):
    preds = np.asarray(preds, dtype=np.float32)
    targets = np.asarray(targets, dtype=np.float32)
    d = np.ascontiguousarray(targets[:, 0])
    e = np.ascontiguousarray(targets[:, 1])

    dmask = np.where(e == 1.0, d, DMASK_FILL).astype(np.float32)
    predsbf = preds.astype(BF16)

    in_maps = []
    for c in range(NCORES):
        jsl = slice(c * JB, (c + 1) * JB)
        dj = np.ascontiguousarray(d[jsl].reshape(NT, 128).T)           # [128, NT]
        djbig = (BIG * dj).astype(np.float32)                          # f32 mult = device rounding
        djcols = np.concatenate([dj, djbig], axis=1)                   # [128, 2NT]
        pj = np.ascontiguousarray(preds[jsl].reshape(NT, 128).T)       # [128, NT]
        pj1 = (pj + np.float32(1.0)).astype(BF16)
        phi = pj.astype(BF16)
        plo = (pj - phi.astype(np.float32)).astype(BF16)
        lhst = np.stack([phi, plo, np.ones_like(phi)], axis=-1)        # [128, NT, 3]
        pcols = np.concatenate([lhst.reshape(128, 3 * NT), pj1], axis=1)
        in_maps.append(
            {
                "dmask": dmask,
                "predsbf": predsbf,
                "djcols": np.ascontiguousarray(djcols),
                "pcols": np.ascontiguousarray(pcols),
            }
        )
    return in_maps


def combine_outputs(preds, results):
    """results: per-core dicts with outj [NCH,NSUB,3,SUB], outra/outrg/outrd."""
    preds = np.asarray(preds, dtype=np.float64)
    S1e = np.zeros(N, dtype=np.float64)
    S0e = np.zeros(N, dtype=np.float64)
    pairs = 0.0
    for res in results:
        outj = np.asarray(res["outj"], dtype=np.float64)
        S1e += (outj[:, :, 0, :] + outj[:, :, 1, :]).reshape(N)
        S0e += outj[:, :, 2, :].reshape(N)
        for k in ("outra", "outrg", "outrd"):
            pairs += float(np.asarray(res[k], dtype=np.float64).sum())
    loss_sum = float(np.sum(S1e + (1.0 - preds) * S0e))
    if pairs > 0:
        out = loss_sum / max(pairs, 1.0)
    else:
        out = 0.0
    return np.float32(out)


def kernel(preds, targets):
    from concourse.bass_utils import run_bass_kernel_spmd

    nc = get_module()
    in_maps = make_in_maps(preds, targets)
    res = run_bass_kernel_spmd(nc, in_maps, core_ids=list(range(NCORES)))
    return combine_outputs(preds, res.results)


# revision 25
# speedup vs baseline: 2.6179x; 2.6179x over previous
"""Trainium2 Bass kernel for nn_RankingLoss (pairwise hinge ranking loss).

reference semantics (N = 8192):
    d = targets[:,0]; e = targets[:,1]
    valid[i,j] = (d[i] < d[j]) & (e[i] == 1)
    hinge[i,j] = relu(1.0 - (p[i] - p[j]))
    loss = sum(valid*hinge) / max(sum(valid), 1)   (0 if no pairs)

Device algorithm (per core, j-axis sharded across 8 cores, both axes sorted
by duration on the host — an O(N log N) relabeling, like causal masking):

  After sorting, [d_i < d_j] is a rank triangle up to exact-tie noise, so
  for an i-chunk strictly below a j-tile's rank range the mask is just e_i,
  and for a chunk strictly above it is 0 (those ops and matmuls are simply
  skipped — ~75% of the pairwise work is provably zero).  Only the chunk
  containing the tile's own ranks (its "diagonal" chunk) computes the exact
  f32 duration compare.

  Layout: partition axis = j (128 per tile; core c's tile t covers sorted
  ranks [1024 t + 128 c, +128) so every core touches all rank levels and the
  load is balanced), free axis = i (4 macro-chunks of 2048).  The i-axis
  vectors are broadcast across partitions with a K=16 TensorE matmul over
  16 host-replicated rows (the sum scales values by exactly 16, which is
  folded into the j-side scalars; 16 rows make the input DMA fast), then one
  engine copy per chunk — keeping every hot-loop dependency a single-engine
  semaphore (walrus fits only one sync wait on LDWEIGHTS).

  We[j,i] = [16 bf16(p_i) < 16 bf16(p_j+1)] * e_i     (e folded via a bf16
            sentinel in the masked preds broadcast; DVE tensor_scalar, 4x)
  A_e[j,i] = [16 dmask_i < 16 d_j]  (dmask = e ? d : 1e6; only on diagonal
            chunks; ScalarE sigmoid(BIG*(d16_j - d16mask_i)), accum_out
            gives the diagonal num_pairs partial)
  J = A_e * We on diagonal chunks (DVE tensor_tensor, bf16 2x); J = We on
            below-chunks (free).
  PSUM += sum_j J * [p_hi_j, p_lo_j, 1]  via TensorE matmuls, col-tiled so a
            whole macro-chunk accumulates in one fresh psum bank
            (p_hi + p_lo = f32 preds split into two bf16 for precision).

  Host: loss_sum = sum_i S1e_i + (1 - p_i) S0e_i (in sorted space),
  num_pairs = sum(diagonal accums) + 128 * sum_t prefix_eones[below(t)]
  (exact integers).  The p-compare runs in bf16: any pair it can misclassify
  has |hinge| <= one bf16 ulp, so loss error stays ~1e-4 relative; the
  duration compare is exact except for saturated-sigmoid boundary pairs
  (|d_i-d_j| < ~1e-7 d) and rank ties exactly at chunk boundaries, both
  O(1e-6) relative.
"""

import numpy as np
import ml_dtypes

N = 8192
NCORES = 8
JB = N // NCORES          # j's per core = 1024
NT = JB // 128            # j-tiles per core = 8
CH = 2048                 # i macro-chunk width
NCH = N // CH             # 4
SUB = 512                 # matmul N / psum bank width (f32)
NSUB = CH // SUB          # 4
BCH = 1024                # broadcast psum chunk width
REP = 16                  # host-replicated rows for the broadcast matmul
BIG = np.float32(1.0e30)
DMASK_FILL = np.float32(1.0e6)   # finite sentinel > any duration
PSENT = np.float32(1.0e30)       # bf16 sentinel > any 16*(p+1)
BF16 = ml_dtypes.bfloat16

_CACHE = {}


def _tile_rank0(c, t):
    """First sorted rank covered by core c's j-tile t."""
    return 1024 * t + 128 * c


def _build_module():
    import concourse.bass as bass
    import concourse.bacc as bacc
    import concourse.tile as tile
    from concourse import mybir

    f32 = mybir.dt.float32
    bf16 = mybir.dt.bfloat16
    Alu = mybir.AluOpType
    Act = mybir.ActivationFunctionType

    nc = bacc.Bacc(trn_type="TRN2")
    t_dm = nc.dram_tensor("dmask16", [REP, N], f32, kind="ExternalInput")
    t_pe = nc.dram_tensor("pebf16", [REP, N], bf16, kind="ExternalInput")
    # djcols: [:, 0:NT] = 16*dj, [:, NT:2NT] = BIG*16*dj, [:, 2NT:3NT] = 16*bf16(p_j+1)
    t_djcols = nc.dram_tensor("djcols", [128, 3 * NT], f32, kind="ExternalInput")
    # pcols: lhst per t, zero-padded to 32 cols ([p_hi|p_lo|1|0...]) so the
    # start=True matmul initializes the full 32-partition psum group.
    t_pcols = nc.dram_tensor("pcols", [128, 32 * NT], bf16, kind="ExternalInput")
    t_outj = nc.dram_tensor("outj", [NCH, NSUB, 3, SUB], f32, kind="ExternalOutput")
    t_outra = nc.dram_tensor("outra", [128, NT], f32, kind="ExternalOutput")

    with tile.TileContext(nc) as tc:
        with (
            tc.tile_pool(name="consts", bufs=1) as consts,
            tc.tile_pool(name="bcast", bufs=1) as bcast,
            tc.tile_pool(name="awork", bufs=3) as awork,
            tc.tile_pool(name="wwork", bufs=4) as wwork,
            tc.tile_pool(name="jwork", bufs=3) as jwork,
            tc.tile_pool(name="stage", bufs=2) as stagep,
            tc.tile_pool(name="scratch", bufs=1) as scratch,
            tc.tile_pool(name="bps", bufs=2, space="PSUM") as bpsp,
            tc.tile_pool(name="acc", bufs=4, space="PSUM") as accp,
        ):
            djcols_s = consts.tile([128, 3 * NT], f32, tag="djcols")
            pcols_s = consts.tile([128, 32 * NT], bf16, tag="pcols")
            dmrows = consts.tile([REP, N], f32, tag="dmrows")
            perows = consts.tile([REP, N], bf16, tag="perows")
            ones_f = consts.tile([REP, 128], f32, tag="ones_f")
            ones_b = consts.tile([REP, 128], bf16, tag="ones_b")
            r_act = consts.tile([128, NT], f32, tag="ract")

            nc.sync.dma_start(djcols_s[:], t_djcols[:])
            nc.sync.dma_start(pcols_s[:], t_pcols[:])
            # Column-split the replicated-row loads: one DMA per macro-chunk
            # so they spread over queues and chunk C's broadcast matmuls wait
            # on exactly one DMA semaphore.
            for b in range(N // BCH):
                csl = slice(b * BCH, (b + 1) * BCH)
                nc.sync.dma_start(dmrows[:, csl], t_dm[:, csl])
                nc.sync.dma_start(perows[:, csl], t_pe[:, csl])
            nc.vector.memset(ones_f[:], 1.0)
            nc.vector.memset(ones_b[:], 1.0)

            # Tiny warm-up copies so the big ops don't accumulate DMA waits.
            warm_a = scratch.tile([128, 1], f32, tag="warm_a")
            warm_v = scratch.tile([128, 1], bf16, tag="warm_v")
            nc.scalar.copy(warm_a[:], djcols_s[:, 0:1])
            nc.vector.tensor_copy(warm_v[:], pcols_s[:, 0:1])

            # Broadcast i-axis vectors across partitions: K=REP PE matmul
            # (ones.T @ rows -> 16x-scaled values in PSUM), engine copy to
            # SBUF.  All d-chunks first (ScalarE copies), then p-chunks (DVE
            # copies), so every psum-slot reuse is one engine semaphore.
            pbc = [
                bcast.tile([128, CH], bf16, tag=f"pbc{C}", name=f"pbc{C}")
                for C in range(NCH)
            ]
            first = True
            for C in range(NCH):
                for h in range(CH // BCH):
                    off = C * CH + h * BCH
                    bp2 = bpsp.tile([128, BCH], f32, tag="bps")
                    if first:
                        # Dummy 1x1 matmuls: advance PE's vector clock past
                        # the memsets and row DMAs one semaphore at a time.
                        for wlhs, wrhs in (
                            (ones_b, ones_b),
                            (ones_f, dmrows),
                            (ones_b, perows),
                        ):
                            nc.tensor.matmul(
                                bp2[0:1, 0:1], wlhs[0:1, 0:1], wrhs[0:1, 0:1],
                                start=True, stop=True,
                            )
                        first = False
                    for s in range(BCH // SUB):
                        nc.tensor.matmul(
                            bp2[:, s * SUB : (s + 1) * SUB],
                            ones_b[:],
                            perows[:, off + s * SUB : off + (s + 1) * SUB],
                            start=True,
                            stop=True,
                        )
                    if h % 2 == 0:
                        nc.vector.tensor_copy(pbc[C][:, h * BCH : (h + 1) * BCH], bp2[:])
                    else:
                        nc.scalar.copy(pbc[C][:, h * BCH : (h + 1) * BCH], bp2[:])

            HB = CH // 2  # 1024: half-chunk; tile t's exact-compare region
            for C in range(NCH):
                ps_c = accp.tile([128, SUB], f32, tag="acc")
                nc.vector.memset(ps_c[:], 0.0)
                # Tiles t with t//2 > C are fully below (rhs = We on the whole
                # chunk); t == 2C owns the lower half (upper half all-zero);
                # t == 2C+1 owns the upper half (lower half is We-only).
                for t in range(2 * C, NT):
                    diag = t // 2 == C
                    even = t % 2 == 0
                    wewidth = HB if (diag and even) else CH
                    we_t = wwork.tile([128, CH], bf16, tag="we")
                    nc.vector.tensor_scalar(
                        we_t[:, :wewidth],
                        pbc[C][:, :wewidth],
                        djcols_s[:, 2 * NT + t : 2 * NT + t + 1],
                        None,
                        Alu.is_lt,
                    )
                    if diag:
                        # Exact duration compare (e-masked) on this tile's
                        # half-chunk; accum_out = num_pairs partial.
                        hsl = slice(0, HB) if even else slice(HB, CH)
                        a_t = awork.tile([128, HB], bf16, tag="a")
                        nc.scalar.activation(
                            a_t[:],
                            dbc[C][:, hsl],
                            Act.Sigmoid,
                            bias=djcols_s[:, NT + t : NT + t + 1],
                            scale=-float(BIG),
                            accum_out=r_act[:, t : t + 1],
                        )
                        j_t = jwork.tile([128, HB], bf16, tag="j")
                        nc.vector.tensor_tensor(
                            j_t[:], a_t[:], we_t[:, hsl], Alu.mult
                        )
                        if even:
                            rhs_by_sub = [j_t[:, 0:SUB], j_t[:, SUB:HB], None, None]
                        else:
                            rhs_by_sub = [
                                we_t[:, 0:SUB],
                                we_t[:, SUB:HB],
                                j_t[:, 0:SUB],
                                j_t[:, SUB:HB],
                            ]
                    else:
                        rhs_by_sub = [
                            we_t[:, s * SUB : (s + 1) * SUB] for s in range(NSUB)
                        ]
                    for s in range(NSUB):
                        if rhs_by_sub[s] is None:
                            continue
                        nc.tensor.matmul(
                            ps_c[32 * s : 32 * s + 3, :],
                            pcols_s[:, 3 * t : 3 * t + 3],
                            rhs_by_sub[s],
                            start=(t == 2 * C + (1 if s >= 2 else 0)),
                            stop=(t == NT - 1),
                            tile_position=(0, 32 * s),
                        )
                st = stagep.tile([128, SUB], f32, tag="st")
                nc.scalar.copy(st[:], ps_c[:])
                for s in range(NSUB):
                    nc.sync.dma_start(t_outj[C, s], st[32 * s : 32 * s + 3, :])

            nc.sync.dma_start(t_outra[:], r_act[:])

    nc.finalize()  # Bacc: legalizes sync waits (event semaphores) + compiles
    return nc


def get_module():
    if "nc" not in _CACHE:
        _CACHE["nc"] = _build_module()
    return _CACHE["nc"]


def _sort_inputs(preds, targets):
    preds = np.asarray(preds, dtype=np.float32)
    targets = np.asarray(targets, dtype=np.float32)
    d = np.ascontiguousarray(targets[:, 0])
    e = np.ascontiguousarray(targets[:, 1])
    order = np.argsort(d, kind="stable")
    return preds[order], d[order], e[order]


def make_in_maps(preds, targets):
    p_s, d_s, e_s = _sort_inputs(preds, targets)

    dmask = np.where(e_s == 1.0, d_s, DMASK_FILL).astype(np.float32)
    pe_masked = np.where(e_s == 1.0, p_s.astype(BF16), PSENT.astype(BF16))
    dmask16 = np.ascontiguousarray(np.tile(dmask, (REP, 1)))
    pebf16 = np.ascontiguousarray(np.tile(pe_masked, (REP, 1)))

    in_maps = []
    for c in range(NCORES):
        dj = np.empty((128, NT), np.float32)
        pj = np.empty((128, NT), np.float32)
        for t in range(NT):
            r0 = _tile_rank0(c, t)
            dj[:, t] = d_s[r0 : r0 + 128]
            pj[:, t] = p_s[r0 : r0 + 128]
        dj16 = (np.float32(REP) * dj).astype(np.float32)   # exact (x16)
        djbig = (BIG * dj16).astype(np.float32)
        pj1_16 = ((pj + np.float32(1.0)).astype(BF16).astype(np.float32)
                  * np.float32(REP)).astype(np.float32)     # exact x16 of bf16, as f32
        djcols = np.concatenate([dj16, djbig, pj1_16], axis=1)
        phi = pj.astype(BF16)
        plo = (pj - phi.astype(np.float32)).astype(BF16)
        lhst = np.stack([phi, plo, np.ones_like(phi)], axis=-1)
        pcols = lhst.reshape(128, 3 * NT)
        in_maps.append(
            {
                "dmask16": dmask16,
                "pebf16": pebf16,
                "djcols": np.ascontiguousarray(djcols),
                "pcols": np.ascontiguousarray(pcols),
            }
        )
    return in_maps


def combine_outputs(preds, targets, results):
    """results: per-core dicts with outj [NCH,NSUB,3,SUB], outra [128,NT]."""
    p_s, d_s, e_s = _sort_inputs(preds, targets)
    p64 = p_s.astype(np.float64)

    S1e = np.zeros(N, dtype=np.float64)
    S0e = np.zeros(N, dtype=np.float64)
    pairs = 0.0
    for res in results:
        outj = np.asarray(res["outj"], dtype=np.float64)
        S1e += (outj[:, :, 0, :] + outj[:, :, 1, :]).reshape(N)
        S0e += outj[:, :, 2, :].reshape(N)
        pairs += float(np.asarray(res["outra"], dtype=np.float64).sum())

    # Below-diagonal num_pairs term: each j of tile t sees all event-i's with
    # rank below its half-chunk boundary 1024*t (the device's exact compare
    # covers [1024 t, 1024 (t+1)) and above is all-zero).
    eones_prefix = np.concatenate([[0.0], np.cumsum(e_s == 1.0)])
    for t in range(NT):
        pairs += NCORES * 128 * float(eones_prefix[1024 * t])

    loss_sum = float(np.sum(S1e + (1.0 - p64) * S0e))
    if pairs > 0:
        out = loss_sum / max(pairs, 1.0)
    else:
        out = 0.0
    return np.float32(out)


def kernel(preds, targets):
    from concourse.bass_utils import run_bass_kernel_spmd

    nc = get_module()
    in_maps = make_in_maps(preds, targets)
    res = run_bass_kernel_spmd(nc, in_maps, core_ids=list(range(NCORES)))
    return combine_outputs(preds, targets, res.results)
